# revision 18
# baseline (speedup 1.0000x reference)
"""Kabsch loss kernel for Trainium2 (8 NeuronCores, data-parallel over batch).

Reference:
    x_c = x - mean_n(x); y_c = y - mean_n(y)
    C = x_c^T y_c  (3x3 per batch);  U,S,Vh = svd(C);  R = U Vh
    loss = mean(|x_c R - y_c|^2)
Since R is orthogonal and tr(R^T C) = tr(S):
    loss = (1/(B*N*3)) * sum_b [ |x_c|_F^2 + |y_c|_F^2 - 2*sum(svdvals(C_b)) ]
The device computes raw per-batch stats (G = x^T y, sum_x, sum_y, ssq_x, ssq_y);
the host centers them and does the 8192 tiny 3x3 SVDs in float64.

Device kernel (bf16 data path, fp32 accumulation), per 128-batch tile:
  - inputs arrive as bf16 via SWDGE cast-DMA (fp32 DRAM -> bf16 SBUF);
    bf16 strided reads are ~1.45x cheaper on ACT than fp32
  - de-interleave (b, (n c)) -> 6 coordinate planes with the per-coordinate
    sums accumulated for free: DVE tensor_scalar(+accum) for x0,
    ACT activation(Identity, accum_out) for the other 5
  - 9 fused product+reduce ops (DVE scalar_tensor_tensor with accum_out)
    on the unit-stride bf16 planes -> G entries (fp32 accumulators)
  - 2 fused square+reduce ops (ACT activation(Square, accum_out)) on the
    raw interleaved tiles -> ssq_x, ssq_y
This keeps DVE ~= ACT ~= 12.5us/tile overlapping the ~9.5us/tile DMA
(122us/core measured; bf16 rounding costs ~8e-6 relative error).
"""

import numpy as np

import jax
from jax.sharding import Mesh, NamedSharding, PartitionSpec

import concourse.bass as bass
import concourse.mybir as mybir
import concourse.tile as tile
from concourse import bass2jax

B, N = 8192, 1024
NCORES = 8
BPC = B // NCORES          # batches per core
P = 128                    # partitions
NTILES = BPC // P          # tiles of 128 batches per core

FP32 = mybir.dt.float32
BF16 = mybir.dt.bfloat16

# stats layout per batch row: [G(9) | sum_x(3) | sum_y(3) | ssq_x | ssq_y]
NSTAT = 17


def _body(nc, x, y):
    st = nc.dram_tensor("stats", (BPC, NSTAT), FP32, kind="ExternalOutput")

    xr = x[:, :, :].rearrange("(t p) n c -> t p (n c)", p=P)
    yr = y[:, :, :].rearrange("(t p) n c -> t p (n c)", p=P)

    MUL = mybir.AluOpType.mult
    ADD = mybir.AluOpType.add

    with tile.TileContext(nc) as tc:
        with (
            tc.tile_pool(name="data", bufs=3) as dpool,
            tc.tile_pool(name="planes", bufs=4) as ppool,
            tc.tile_pool(name="scr", bufs=2) as spool,
            tc.tile_pool(name="stats", bufs=4) as stpool,
        ):
            for t in range(NTILES):
                # SWDGE cast-DMA: fp32 DRAM -> bf16 SBUF (halves SBUF traffic,
                # and bf16 strided reads are much cheaper on ACT). One [x|y]
                # buffer so a single Square+accum covers ssq_x + ssq_y.
                xy = dpool.tile([P, 2 * N * 3], BF16, tag="xy")
                xt = xy[:, 0 : N * 3]
                yt = xy[:, N * 3 : 2 * N * 3]
                nc.gpsimd.dma_start(out=xt, in_=xr[t])
                nc.gpsimd.dma_start(out=yt, in_=yr[t])

                xv = xt.rearrange("p (n c) -> p n c", c=3)
                yv = yt.rearrange("p (n c) -> p n c", c=3)

                sdve = stpool.tile([P, NSTAT], FP32, tag="sdve")

                planes = ppool.tile([P, 6 * N], BF16, tag="planes")
                pv = planes[:, :].rearrange("p (c n) -> p c n", c=6)

                # --- de-interleave + per-coordinate sums (fused) ---
                # y planes first on ACT (the early products need them),
                # x0 on DVE via tensor_scalar(+accum), x1/x2 on ACT.
                for j in range(3):
                    nc.scalar.activation(
                        out=pv[:, 3 + j, :], in_=yv[:, :, j],
                        func=mybir.ActivationFunctionType.Identity,
                        accum_out=sdve[:, 12 + j : 13 + j],
                    )
                nc.vector.tensor_scalar(
                    out=pv[:, 0, :], in0=xv[:, :, 0],
                    scalar1=1.0, scalar2=0.0, op0=MUL, op1=ADD,
                    accum_out=sdve[:, 9:10],
                )
                for i in (1, 2):
                    nc.scalar.activation(
                        out=pv[:, i, :], in_=xv[:, :, i],
                        func=mybir.ActivationFunctionType.Identity,
                        accum_out=sdve[:, 9 + i : 10 + i],
                    )

                # --- G_ij = sum_n x_i y_j : fused product+reduce on DVE ---
                # x-major inner order so rows 0/1 (whose planes come from the
                # fast DVE path) run while ACT finishes x2 and the squares.
                prod = spool.tile([P, N], BF16, tag="prod")
                for j in range(3):
                    for i in range(3):
                        k = 3 * i + j
                        nc.vector.scalar_tensor_tensor(
                            out=prod[:, :], in0=pv[:, i, :], scalar=1.0,
                            in1=pv[:, 3 + j, :], op0=MUL, op1=MUL,
                            accum_out=sdve[:, k : k + 1],
                        )

                # --- combined ssq_x + ssq_y via ONE ACT Square(+accum) over
                #     the contiguous [x | y] buffer (only the sum is needed) ---
                scrq = spool.tile([P, 2 * N * 3], BF16, tag="act_scr")
                nc.scalar.activation(
                    out=scrq[:, :], in_=xy[:, :],
                    func=mybir.ActivationFunctionType.Square,
                    accum_out=sdve[:, 15:16],
                )
                nc.gpsimd.memset(sdve[:, 16:17], 0.0)

                nc.sync.dma_start(out=st[t * P : (t + 1) * P, :], in_=sdve[:, :])
    return st


VERSION = 3
_CACHE = {}


def _get_runner():
    if "runner" not in _CACHE:
        jitted = bass2jax.bass_jit(_body)
        out_specs = PartitionSpec("core")
        devices = jax.devices()[:NCORES]
        mesh = Mesh(np.asarray(devices), ("core",))
        f = bass2jax.bass_shard_map(
            jitted,
            mesh=mesh,
            in_specs=(PartitionSpec("core"), PartitionSpec("core")),
            out_specs=out_specs,
        )
        _CACHE["runner"] = (f, mesh)
    return _CACHE["runner"]


def _host_finish(stats: np.ndarray) -> np.ndarray:
    s = stats.astype(np.float64)
    nb = s.shape[0]
    G = s[:, 0:9].reshape(nb, 3, 3)
    sx = s[:, 9:12]
    sy = s[:, 12:15]
    ssq_tot = s[:, 15]  # ssq_x + ssq_y combined (slot 16 is zero)
    C = G - sx[:, :, None] * sy[:, None, :] / N
    nuc = np.linalg.svd(C, compute_uv=False).sum(1)
    ss_c = ssq_tot - (sx**2).sum(1) / N - (sy**2).sum(1) / N
    loss = (ss_c - 2.0 * nuc).sum() / (nb * N * 3)
    return np.asarray(loss, dtype=np.float32)


def kernel(x, y):
    f, _ = _get_runner()
    x = np.ascontiguousarray(np.asarray(x, dtype=np.float32))
    y = np.ascontiguousarray(np.asarray(y, dtype=np.float32))
    out = jax.block_until_ready(f(x, y))
    return _host_finish(np.asarray(out))


def bench(x, y, iters=10):
    import time

    f, mesh = _get_runner()
    sh = NamedSharding(mesh, PartitionSpec("core"))
    xd = jax.device_put(np.asarray(x, dtype=np.float32), sh)
    yd = jax.device_put(np.asarray(y, dtype=np.float32), sh)
    jax.block_until_ready(f(xd, yd))  # warm up / compile
    times = []
    for _ in range(iters):
        t0 = time.perf_counter()
        jax.block_until_ready(f(xd, yd))
        times.append(time.perf_counter() - t0)
    return times


# revision 19
# speedup vs baseline: 1.0253x; 1.0253x over previous
"""Kabsch loss kernel for Trainium2 (8 NeuronCores, data-parallel over batch).

Reference:
    x_c = x - mean_n(x); y_c = y - mean_n(y)
    C = x_c^T y_c  (3x3 per batch);  U,S,Vh = svd(C);  R = U Vh
    loss = mean(|x_c R - y_c|^2)
Since R is orthogonal and tr(R^T C) = tr(S):
    loss = (1/(B*N*3)) * sum_b [ |x_c|_F^2 + |y_c|_F^2 - 2*sum(svdvals(C_b)) ]
The device computes raw per-batch stats (G = x^T y, sum_x, sum_y, ssq_x, ssq_y);
the host centers them and does the 8192 tiny 3x3 SVDs in float64.

Device kernel (bf16 data path, fp32 accumulation), per 128-batch tile:
  - inputs arrive as bf16 via SWDGE cast-DMA (fp32 DRAM -> bf16 SBUF);
    bf16 strided reads are ~1.45x cheaper on ACT than fp32
  - de-interleave (b, (n c)) -> 6 coordinate planes with the per-coordinate
    sums accumulated for free: DVE tensor_scalar(+accum) for x0,
    ACT activation(Identity, accum_out) for the other 5
  - 9 fused product+reduce ops (DVE scalar_tensor_tensor with accum_out)
    on the unit-stride bf16 planes -> G entries (fp32 accumulators)
  - 2 fused square+reduce ops (ACT activation(Square, accum_out)) on the
    raw interleaved tiles -> ssq_x, ssq_y
This keeps DVE ~= ACT ~= 12.5us/tile overlapping the ~9.5us/tile DMA
(122us/core measured; bf16 rounding costs ~8e-6 relative error).
"""

import numpy as np

import jax
from jax.sharding import Mesh, NamedSharding, PartitionSpec

import concourse.bass as bass
import concourse.mybir as mybir
import concourse.tile as tile
from concourse import bass2jax

B, N = 8192, 1024
NCORES = 8
BPC = B // NCORES          # batches per core
P = 128                    # partitions
NTILES = BPC // P          # tiles of 128 batches per core

FP32 = mybir.dt.float32
BF16 = mybir.dt.bfloat16

# stats layout per batch row: [G(9) | sum_x(3) | sum_y(3) | ssq_x | ssq_y]
NSTAT = 17


def _body(nc, x, y):
    st = nc.dram_tensor("stats", (BPC, NSTAT), FP32, kind="ExternalOutput")

    xr = x[:, :, :].rearrange("(t p) n c -> t p (n c)", p=P)
    yr = y[:, :, :].rearrange("(t p) n c -> t p (n c)", p=P)

    MUL = mybir.AluOpType.mult
    ADD = mybir.AluOpType.add

    with tile.TileContext(nc) as tc:
        with (
            tc.tile_pool(name="data", bufs=2) as dpool,
            tc.tile_pool(name="planes", bufs=3) as ppool,
            tc.tile_pool(name="scr", bufs=2) as spool,
            tc.tile_pool(name="stats", bufs=4) as stpool,
        ):
            for t in range(NTILES):
                # SWDGE cast-DMA: fp32 DRAM -> bf16 SBUF (halves SBUF traffic,
                # and bf16 strided reads are much cheaper on ACT)
                xt = dpool.tile([P, N * 3], BF16, tag="xt")
                yt = dpool.tile([P, N * 3], BF16, tag="yt")
                nc.gpsimd.dma_start(out=xt[:, :], in_=xr[t])
                nc.gpsimd.dma_start(out=yt[:, :], in_=yr[t])

                xv = xt[:, :].rearrange("p (n c) -> p n c", c=3)
                yv = yt[:, :].rearrange("p (n c) -> p n c", c=3)

                sdve = stpool.tile([P, NSTAT], FP32, tag="sdve")

                planes = ppool.tile([P, 6 * N], BF16, tag="planes")
                pv = planes[:, :].rearrange("p (c n) -> p c n", c=6)

                # --- de-interleave + per-coordinate sums (fused) ---
                # y planes first on ACT (the early products need them),
                # x0 on DVE via tensor_scalar(+accum), x1/x2 on ACT.
                for j in range(3):
                    nc.scalar.activation(
                        out=pv[:, 3 + j, :], in_=yv[:, :, j],
                        func=mybir.ActivationFunctionType.Identity,
                        accum_out=sdve[:, 12 + j : 13 + j],
                    )
                nc.vector.tensor_scalar(
                    out=pv[:, 0, :], in0=xv[:, :, 0],
                    scalar1=1.0, scalar2=0.0, op0=MUL, op1=ADD,
                    accum_out=sdve[:, 9:10],
                )
                for i in (1, 2):
                    nc.scalar.activation(
                        out=pv[:, i, :], in_=xv[:, :, i],
                        func=mybir.ActivationFunctionType.Identity,
                        accum_out=sdve[:, 9 + i : 10 + i],
                    )

                # --- G_ij = sum_n x_i y_j : fused product+reduce on DVE ---
                # x-major inner order so rows 0/1 (whose planes come from the
                # fast DVE path) run while ACT finishes x2 and the squares.
                prod = spool.tile([P, N], BF16, tag="prod")
                for j in range(3):
                    for i in range(3):
                        k = 3 * i + j
                        nc.vector.scalar_tensor_tensor(
                            out=prod[:, :], in0=pv[:, i, :], scalar=1.0,
                            in1=pv[:, 3 + j, :], op0=MUL, op1=MUL,
                            accum_out=sdve[:, k : k + 1],
                        )

                # --- ssq via ACT Square(+accum) on the raw interleaved tiles
                #     (independent of everything; fills ACT during products) ---
                scrx = spool.tile([P, N * 3], BF16, tag="act_scr")
                nc.scalar.activation(
                    out=scrx[:, :], in_=xt[:, :],
                    func=mybir.ActivationFunctionType.Square,
                    accum_out=sdve[:, 15:16],
                )
                scry = spool.tile([P, N * 3], BF16, tag="act_scr")
                nc.scalar.activation(
                    out=scry[:, :], in_=yt[:, :],
                    func=mybir.ActivationFunctionType.Square,
                    accum_out=sdve[:, 16:17],
                )

                nc.sync.dma_start(out=st[t * P : (t + 1) * P, :], in_=sdve[:, :])
    return st


VERSION = 3
_CACHE = {}


def _get_runner():
    if "runner" not in _CACHE:
        jitted = bass2jax.bass_jit(_body)
        out_specs = PartitionSpec("core")
        devices = jax.devices()[:NCORES]
        mesh = Mesh(np.asarray(devices), ("core",))
        f = bass2jax.bass_shard_map(
            jitted,
            mesh=mesh,
            in_specs=(PartitionSpec("core"), PartitionSpec("core")),
            out_specs=out_specs,
        )
        _CACHE["runner"] = (f, mesh)
    return _CACHE["runner"]


def _host_finish(stats: np.ndarray) -> np.ndarray:
    s = stats.astype(np.float64)
    nb = s.shape[0]
    G = s[:, 0:9].reshape(nb, 3, 3)
    sx = s[:, 9:12]
    sy = s[:, 12:15]
    ssx = s[:, 15]
    ssy = s[:, 16]
    C = G - sx[:, :, None] * sy[:, None, :] / N
    nuc = np.linalg.svd(C, compute_uv=False).sum(1)
    ssxc = ssx - (sx**2).sum(1) / N
    ssyc = ssy - (sy**2).sum(1) / N
    loss = (ssxc + ssyc - 2.0 * nuc).sum() / (nb * N * 3)
    return np.asarray(loss, dtype=np.float32)


def kernel(x, y):
    f, _ = _get_runner()
    x = np.ascontiguousarray(np.asarray(x, dtype=np.float32))
    y = np.ascontiguousarray(np.asarray(y, dtype=np.float32))
    out = jax.block_until_ready(f(x, y))
    return _host_finish(np.asarray(out))


def bench(x, y, iters=10):
    import time

    f, mesh = _get_runner()
    sh = NamedSharding(mesh, PartitionSpec("core"))
    xd = jax.device_put(np.asarray(x, dtype=np.float32), sh)
    yd = jax.device_put(np.asarray(y, dtype=np.float32), sh)
    jax.block_until_ready(f(xd, yd))  # warm up / compile
    times = []
    for _ in range(iters):
        t0 = time.perf_counter()
        jax.block_until_ready(f(xd, yd))
        times.append(time.perf_counter() - t0)
    return times


# revision 20
# speedup vs baseline: 1.0341x; 1.0086x over previous
"""Kabsch loss kernel for Trainium2 (8 NeuronCores, data-parallel over batch).

Reference:
    x_c = x - mean_n(x); y_c = y - mean_n(y)
    C = x_c^T y_c  (3x3 per batch);  U,S,Vh = svd(C);  R = U Vh
    loss = mean(|x_c R - y_c|^2)
Since R is orthogonal and tr(R^T C) = tr(S):
    loss = (1/(B*N*3)) * sum_b [ |x_c|_F^2 + |y_c|_F^2 - 2*sum(svdvals(C_b)) ]
The device computes raw per-batch stats (G = x^T y, sum_x, sum_y, ssq_x, ssq_y);
the host centers them and does the 8192 tiny 3x3 SVDs in float64.

Device kernel (bf16 data path, fp32 accumulation), per 128-batch tile:
  - inputs arrive as bf16 via SWDGE cast-DMA (fp32 DRAM -> bf16 SBUF);
    bf16 strided reads are ~1.45x cheaper on ACT than fp32
  - de-interleave (b, (n c)) -> 6 coordinate planes with the per-coordinate
    sums accumulated for free: DVE tensor_scalar(+accum) for x0,
    ACT activation(Identity, accum_out) for the other 5
  - 9 fused product+reduce ops (DVE scalar_tensor_tensor with accum_out)
    on the unit-stride bf16 planes -> G entries (fp32 accumulators)
  - 2 fused square+reduce ops (ACT activation(Square, accum_out)) on the
    raw interleaved tiles -> ssq_x, ssq_y
This keeps DVE ~= ACT ~= 12.5us/tile overlapping the ~9.5us/tile DMA
(122us/core measured; bf16 rounding costs ~8e-6 relative error).
"""

import numpy as np

import jax
from jax.sharding import Mesh, NamedSharding, PartitionSpec

import concourse.bass as bass
import concourse.mybir as mybir
import concourse.tile as tile
from concourse import bass2jax

B, N = 8192, 1024
NCORES = 8
BPC = B // NCORES          # batches per core
P = 128                    # partitions
NTILES = BPC // P          # tiles of 128 batches per core

FP32 = mybir.dt.float32
BF16 = mybir.dt.bfloat16

# stats layout per batch row: [G(9) | sum_x(3) | sum_y(3) | ssq_x | ssq_y]
NSTAT = 17


def _body(nc, x, y):
    st = nc.dram_tensor("stats", (BPC, NSTAT), FP32, kind="ExternalOutput")

    xr = x[:, :, :].rearrange("(t p) n c -> t p (n c)", p=P)
    yr = y[:, :, :].rearrange("(t p) n c -> t p (n c)", p=P)

    MUL = mybir.AluOpType.mult
    ADD = mybir.AluOpType.add

    with tile.TileContext(nc) as tc:
        with (
            tc.tile_pool(name="data", bufs=3) as dpool,
            tc.tile_pool(name="planes", bufs=3) as ppool,
            tc.tile_pool(name="scr", bufs=2) as spool,
            tc.tile_pool(name="stats", bufs=4) as stpool,
        ):
            for t in range(NTILES):
                # SWDGE cast-DMA: fp32 DRAM -> bf16 SBUF (halves SBUF traffic,
                # and bf16 strided reads are much cheaper on ACT)
                xt = dpool.tile([P, N * 3], BF16, tag="xt")
                yt = dpool.tile([P, N * 3], BF16, tag="yt")
                nc.gpsimd.dma_start(out=xt[:, :], in_=xr[t])
                nc.gpsimd.dma_start(out=yt[:, :], in_=yr[t])

                xv = xt[:, :].rearrange("p (n c) -> p n c", c=3)
                yv = yt[:, :].rearrange("p (n c) -> p n c", c=3)

                sdve = stpool.tile([P, NSTAT], FP32, tag="sdve")

                planes = ppool.tile([P, 6 * N], BF16, tag="planes")
                pv = planes[:, :].rearrange("p (c n) -> p c n", c=6)

                # --- de-interleave + per-coordinate sums (fused) ---
                # y planes first on ACT (the early products need them),
                # x0 on DVE via tensor_scalar(+accum), x1/x2 on ACT.
                for j in range(3):
                    nc.scalar.activation(
                        out=pv[:, 3 + j, :], in_=yv[:, :, j],
                        func=mybir.ActivationFunctionType.Identity,
                        accum_out=sdve[:, 12 + j : 13 + j],
                    )
                nc.vector.tensor_scalar(
                    out=pv[:, 0, :], in0=xv[:, :, 0],
                    scalar1=1.0, scalar2=0.0, op0=MUL, op1=ADD,
                    accum_out=sdve[:, 9:10],
                )
                for i in (1, 2):
                    nc.scalar.activation(
                        out=pv[:, i, :], in_=xv[:, :, i],
                        func=mybir.ActivationFunctionType.Identity,
                        accum_out=sdve[:, 9 + i : 10 + i],
                    )

                # --- G_ij = sum_n x_i y_j : fused product+reduce on DVE ---
                # x-major inner order so rows 0/1 (whose planes come from the
                # fast DVE path) run while ACT finishes x2 and the squares.
                prod = spool.tile([P, N], BF16, tag="prod")
                for j in range(3):
                    for i in range(3):
                        k = 3 * i + j
                        nc.vector.scalar_tensor_tensor(
                            out=prod[:, :], in0=pv[:, i, :], scalar=1.0,
                            in1=pv[:, 3 + j, :], op0=MUL, op1=MUL,
                            accum_out=sdve[:, k : k + 1],
                        )

                # --- ssq via ACT Square(+accum) on the raw interleaved tiles
                #     (independent of everything; fills ACT during products) ---
                scrx = spool.tile([P, N * 3], BF16, tag="act_scr")
                nc.scalar.activation(
                    out=scrx[:, :], in_=xt[:, :],
                    func=mybir.ActivationFunctionType.Square,
                    accum_out=sdve[:, 15:16],
                )
                scry = spool.tile([P, N * 3], BF16, tag="act_scr")
                nc.scalar.activation(
                    out=scry[:, :], in_=yt[:, :],
                    func=mybir.ActivationFunctionType.Square,
                    accum_out=sdve[:, 16:17],
                )

                nc.sync.dma_start(out=st[t * P : (t + 1) * P, :], in_=sdve[:, :])
    return st


VERSION = 3
_CACHE = {}


def _get_runner():
    if "runner" not in _CACHE:
        jitted = bass2jax.bass_jit(_body)
        out_specs = PartitionSpec("core")
        devices = jax.devices()[:NCORES]
        mesh = Mesh(np.asarray(devices), ("core",))
        f = bass2jax.bass_shard_map(
            jitted,
            mesh=mesh,
            in_specs=(PartitionSpec("core"), PartitionSpec("core")),
            out_specs=out_specs,
        )
        _CACHE["runner"] = (f, mesh)
    return _CACHE["runner"]


def _host_finish(stats: np.ndarray) -> np.ndarray:
    s = stats.astype(np.float64)
    nb = s.shape[0]
    G = s[:, 0:9].reshape(nb, 3, 3)
    sx = s[:, 9:12]
    sy = s[:, 12:15]
    ssx = s[:, 15]
    ssy = s[:, 16]
    C = G - sx[:, :, None] * sy[:, None, :] / N
    nuc = np.linalg.svd(C, compute_uv=False).sum(1)
    ssxc = ssx - (sx**2).sum(1) / N
    ssyc = ssy - (sy**2).sum(1) / N
    loss = (ssxc + ssyc - 2.0 * nuc).sum() / (nb * N * 3)
    return np.asarray(loss, dtype=np.float32)


def kernel(x, y):
    f, _ = _get_runner()
    x = np.ascontiguousarray(np.asarray(x, dtype=np.float32))
    y = np.ascontiguousarray(np.asarray(y, dtype=np.float32))
    out = jax.block_until_ready(f(x, y))
    return _host_finish(np.asarray(out))


def bench(x, y, iters=10):
    import time

    f, mesh = _get_runner()
    sh = NamedSharding(mesh, PartitionSpec("core"))
    xd = jax.device_put(np.asarray(x, dtype=np.float32), sh)
    yd = jax.device_put(np.asarray(y, dtype=np.float32), sh)
    jax.block_until_ready(f(xd, yd))  # warm up / compile
    times = []
    for _ in range(iters):
        t0 = time.perf_counter()
        jax.block_until_ready(f(xd, yd))
        times.append(time.perf_counter() - t0)
    return times
